# revision 1
# baseline (speedup 1.0000x reference)
"""Trainium2 Bass kernel for nn_DistanceProbe.

Computes, for batch [B=8, S=2048, H=768] and proj [H=768, R=768]:
    t  = batch @ proj                      # [B, S, R]
    d2 = relu(||t_i||^2 + ||t_j||^2 - 2 t_i . t_j)   # [B, S, S]

Sharding: data-parallel over B across the 8 NeuronCores (one batch
element per core). Each core receives its batch slice pre-transposed
(xT = batch[b].T, [H, S]) so the contraction dim H lands on SBUF
partitions without any on-device transpose.

Per-core device algorithm (all matmuls in float32r = full-rate fp32):
  1. tT[r, s]   = sum_h proj[h, r] * xT[h, s]        (PE, K=H)
  2. sq[s]      = sum_r tT[r, s]^2                   (DVE square + ones-matmul)
  3. psum[i, j] = sum_r tT[r, i] * tT[r, j]          (PE, K=R)
  4. out[i, j]  = relu(-2*psum + sq_j + sq_i)        (DVE stt + ACT relu w/ bias)

`reps` repeats the whole body inside one NEFF (used by test.py to
measure steady-state HW time by differencing two rep counts).
"""

import numpy as np

import concourse.bass as bass
import concourse.tile as tile
from concourse import bacc
from concourse import masks
from concourse import mybir
from concourse.bass_utils import run_bass_kernel_spmd

B, S, H, R = 8, 2048, 768, 768
N_CORES = 8
P = 128          # SBUF partitions
NC_ = 512        # matmul moving free dim (one PSUM bank of fp32)
HT = H // P      # 6  k-tiles over H
RT = R // P      # 6  k-tiles over R
IT = S // P      # 16 output row tiles
SC = S // NC_    # 4  512-wide column chunks

F32 = mybir.dt.float32


def build_nc(mm_dtype=mybir.dt.float32r, reps=1, symmetric=True):
    nc = bacc.Bacc("TRN2", target_bir_lowering=False, debug=False,
                   num_devices=N_CORES)

    xT_d = nc.dram_tensor("xT", [H, S], mm_dtype, kind="ExternalInput")
    pj_d = nc.dram_tensor("proj", [H, R], mm_dtype, kind="ExternalInput")
    out_d = nc.dram_tensor("out", [S, S], F32, kind="ExternalOutput")

    with tile.TileContext(nc) as tc:
        with tc.tile_pool(name="persist", bufs=1) as sb, \
             tc.tile_pool(name="stage", bufs=4) as stg, \
             tc.tile_pool(name="pmm", bufs=2, space="PSUM") as pmm, \
             tc.tile_pool(name="psq", bufs=1, space="PSUM") as psq, \
             tc.tile_pool(name="pd", bufs=3, space="PSUM") as pdp:

            xT_sb = [sb.tile([P, S], mm_dtype, name=f"xT{i}", tag=f"xT{i}")
                     for i in range(HT)]
            pj_sb = [sb.tile([P, R], mm_dtype, name=f"pj{i}", tag=f"pj{i}")
                     for i in range(HT)]
            tT_sb = [sb.tile([P, S], mm_dtype, name=f"tT{i}", tag=f"tT{i}")
                     for i in range(RT)]
            sqj = sb.tile([P, S], F32, name="sqj", tag="sqj")
            sqrow = sb.tile([1, S], mm_dtype, name="sqrow", tag="sqrow")
            sqrow_f = sb.tile([1, S], F32, name="sqrow_f", tag="sqrowf")
            sqcol = sb.tile([P, IT], F32, name="sqcol", tag="sqcol")
            ones_col = sb.tile([P, 1], mm_dtype, name="ones_col", tag="onc")
            ones_row = sb.tile([1, P], mm_dtype, name="ones_row", tag="onr")
            onesf_col = sb.tile([P, 1], F32, name="onesf_col", tag="onfc")
            onesf_row = sb.tile([1, P], F32, name="onesf_row", tag="onfr")

            nc.vector.memset(onesf_col[:], 1.0)
            nc.vector.memset(onesf_row[:], 1.0)
            nc.vector.tensor_copy(ones_col[:], onesf_col[:])
            nc.vector.tensor_copy(ones_row[:], onesf_row[:])
            if symmetric:
                ident = sb.tile([P, P], F32, name="ident", tag="ident")
                masks.make_identity(nc, ident[:])

            def emit_body():
                # loads: proj first (every matmul group needs all of it)
                for ht in range(HT):
                    nc.sync.dma_start(pj_sb[ht][:],
                                      pj_d[ht * P:(ht + 1) * P, :])
                for sc in range(SC):
                    for ht in range(HT):
                        nc.sync.dma_start(
                            xT_sb[ht][:, sc * NC_:(sc + 1) * NC_],
                            xT_d[ht * P:(ht + 1) * P, sc * NC_:(sc + 1) * NC_])

                # phase B: tT = projT @ x  (tT[r, s]); squares and the
                # sq row-reduction are interleaved per column chunk so the
                # DVE squares overlap the next chunk's PE matmuls
                for sc in range(SC):
                    sq_acc = stg.tile([P, NC_], mm_dtype, name="sq_acc",
                                      tag="sqacc", bufs=2)
                    for rt in range(RT):
                        pt = pmm.tile([P, NC_], F32, name="pt", tag="pt")
                        for ht in range(HT):
                            nc.tensor.matmul(
                                pt[:],
                                pj_sb[ht][:, rt * P:(rt + 1) * P],
                                xT_sb[ht][:, sc * NC_:(sc + 1) * NC_],
                                start=(ht == 0), stop=(ht == HT - 1))
                        nc.scalar.copy(tT_sb[rt][:, sc * NC_:(sc + 1) * NC_],
                                       pt[:])
                        tch = tT_sb[rt][:, sc * NC_:(sc + 1) * NC_]
                        if rt == 0:
                            nc.vector.tensor_mul(sq_acc[:], tch, tch)
                        else:
                            sq_t = stg.tile([P, NC_], mm_dtype, name="sq_t",
                                            tag="sqtmp", bufs=2)
                            nc.vector.tensor_mul(sq_t[:], tch, tch)
                            nc.vector.tensor_add(sq_acc[:], sq_acc[:],
                                                 sq_t[:])
                    sq_ps = psq.tile([1, NC_], F32, name="sq_ps", tag="sq")
                    nc.tensor.matmul(sq_ps[:], ones_col[:], sq_acc[:],
                                     start=True, stop=True)
                    nc.vector.tensor_copy(sqrow[0:1, sc * NC_:(sc + 1) * NC_],
                                          sq_ps[:])
                    nc.vector.tensor_copy(
                        sqrow_f[0:1, sc * NC_:(sc + 1) * NC_], sq_ps[:])

                # sq broadcast across partitions (ones_row^T @ sqrow)
                for sc in range(SC):
                    bc = pmm.tile([P, NC_], F32, name="bc", tag="pt")
                    nc.tensor.matmul(bc[:], ones_row[:],
                                     sqrow[0:1, sc * NC_:(sc + 1) * NC_],
                                     start=True, stop=True)
                    nc.vector.tensor_copy(sqj[:, sc * NC_:(sc + 1) * NC_],
                                          bc[:])

                # sq column form: 16x PE transpose of [1,128] slices
                for it in range(IT):
                    tp = pmm.tile([P, 1], F32, name="tp", tag="pt")
                    nc.tensor.transpose(tp[:],
                                        sqrow_f[0:1, it * P:(it + 1) * P],
                                        onesf_row[0:1, 0:1])
                    nc.vector.tensor_copy(sqcol[:, it:it + 1], tp[:])

                # phase D: dots + fused epilogue (jc-major so mirror
                # chunks batch 4 consecutive source rows)
                def emit_tile(it, jc, strip):
                    j0 = max(jc * NC_, it * P) if symmetric else jc * NC_
                    w = (jc + 1) * NC_ - j0
                    off = 0
                    if 0 < w < 256:
                        # sub-256 f32r matmuls run at 1/4 rate; widen
                        # leftward and discard the overlap columns
                        off = 256 - w
                        j0 -= off
                        w = 256
                    pd = pdp.tile([P, w], F32, name="pd", tag="pd")
                    for rt in range(RT):
                        nc.tensor.matmul(
                            pd[:],
                            tT_sb[rt][:, it * P:(it + 1) * P],
                            tT_sb[rt][:, j0:j0 + w],
                            start=(rt == 0), stop=(rt == RT - 1))
                    jv = j0 + off      # first valid output column
                    wv = w - off
                    st = stg.tile([P, wv], F32, name="st", tag="st", bufs=3)
                    nc.vector.scalar_tensor_tensor(
                        st[:], pd[:, off:w], -2.0,
                        sqj[:, jv:jv + wv],
                        mybir.AluOpType.mult, mybir.AluOpType.add)
                    st2 = stg.tile([P, wv], F32, name="st2", tag="st2",
                                   bufs=11)
                    nc.scalar.activation(
                        st2[:], st[:], mybir.ActivationFunctionType.Relu,
                        bias=sqcol[:, it:it + 1], scale=1.0)
                    nc.sync.dma_start(
                        out_d[it * P:(it + 1) * P, jv:jv + wv], st2[:])
                    strip[it] = (st2, jv)

                def flush_group(jc, it0, it1, strip):
                    # mirror blocks (it, jt) -> (jt, it) for it in
                    # [it0, it1], one [128, <=512] chunk per dest row jt
                    for jt in range(jc * (NC_ // P), (jc + 1) * (NC_ // P)):
                        its = [it for it in range(it0, it1 + 1) if it < jt]
                        if not its:
                            continue
                        cw = len(its) * P
                        mp = pmm.tile([P, cw], F32, name="mp", tag="mp",
                                      bufs=2)
                        for k, it in enumerate(its):
                            st2_t, jv_t = strip[it]
                            nc.tensor.transpose(
                                mp[:, k * P:(k + 1) * P],
                                st2_t[:, jt * P - jv_t:jt * P - jv_t + P],
                                ident[:])
                        mir = stg.tile([P, cw], F32, name="mir", tag="mir",
                                       bufs=6)
                        nc.scalar.copy(mir[:], mp[:])
                        nc.sync.dma_start(
                            out_d[jt * P:(jt + 1) * P,
                                  its[0] * P:(its[0] + len(its)) * P],
                            mir[:])

                if symmetric:
                    for jc in reversed(range(SC)):
                        maxit = jc * (NC_ // P) + (NC_ // P) - 1
                        strip = {}
                        groups = []
                        for it in range(0, maxit + 1):
                            emit_tile(it, jc, strip)
                            if it % 4 == 3 or it == maxit:
                                groups.append((it - it % 4, it))
                            # flush with one-group delay so PE never waits
                            # on this tile's DVE/ACT epilogue
                            if len(groups) > 1:
                                g = groups.pop(0)
                                flush_group(jc, g[0], g[1], strip)
                        for g in groups:
                            flush_group(jc, g[0], g[1], strip)
                else:
                    strip = {}
                    for it in range(IT):
                        for jc in range(SC):
                            emit_tile(it, jc, strip)

            for _ in range(reps):
                emit_body()

    nc.finalize()
    return nc


_NC_CACHE = {}


def get_nc(mm_dtype=mybir.dt.float32r, reps=1, symmetric=True):
    key = (str(mm_dtype), reps, symmetric)
    if key not in _NC_CACHE:
        _NC_CACHE[key] = build_nc(mm_dtype, reps, symmetric)
    return _NC_CACHE[key]


def make_in_maps(batch, proj):
    proj = np.ascontiguousarray(proj, dtype=np.float32)
    return [
        {"xT": np.ascontiguousarray(batch[b].T, dtype=np.float32),
         "proj": proj}
        for b in range(B)
    ]


def kernel(batch, proj):
    assert batch.shape == (B, S, H) and proj.shape == (H, R)
    nc = get_nc()
    in_maps = make_in_maps(batch, proj)
    res = run_bass_kernel_spmd(nc, in_maps, core_ids=list(range(N_CORES)))
    out = np.stack([res.results[b]["out"] for b in range(B)], axis=0)
    return out.astype(np.float32, copy=False)



# revision 4
# speedup vs baseline: 1.5953x; 1.5953x over previous
"""Trainium2 Bass kernel for nn_DistanceProbe.

Computes, for batch [B=8, S=2048, H=768] and proj [H=768, R=768]:
    t  = batch @ proj                      # [B, S, R]
    d2 = relu(||t_i||^2 + ||t_j||^2 - 2 t_i . t_j)   # [B, S, S]

Sharding: data-parallel over B across the 8 NeuronCores (one batch
element per core).

Numerics/performance strategy (validated vs reference in fp8 numpy sim,
max-abs/scale err ~1.3e-2 < 2e-2 gate):
  * Host splits each input into hi/lo fp8e4 pairs: x ~= xh + xl,
    proj ~= ph + pl (residual quantization, ~0.2% relative).
  * Projection t' = xh@ph + xl@ph + xh@pl on PE as fp8e4 DoubleRow
    matmuls (0.5 cyc/row: 2x bf16 rate). Dropped xl@pl term ~0.1%.
  * t' is quantized to fp8e4 (q) by the ACT engine; the SxS Gram matrix
    dots = q.T q runs as fp8e4 DoubleRow matmuls.
  * sq_i = dots_ii is read out of the diagonal-containing Gram tiles
    (identity mask + free-axis reduce on DVE; ones-matmul rebroadcast
    for the row form) => bitwise-consistent with dots, so the relu
    clamp and the zero diagonal are exact in fp8 arithmetic.
  * Epilogue relu(-2*dots + sq_j + sq_i) is two elementwise passes
    (scalar_tensor_tensor; then +bias relu) distributed across
    DVE/ACT/Pool; output written bf16 (lossless host upcast to f32).
  * Emission is chunk-pipelined: Gram wave c is interleaved one chunk
    behind the projection matmuls so every engine streams.

`reps` repeats the whole body inside one NEFF (used by test.py to
measure steady-state HW time by differencing two rep counts).
"""

import numpy as np
import ml_dtypes

import concourse.bass as bass
import concourse.tile as tile
from concourse import bacc
from concourse import masks
from concourse import mybir
from concourse.alu_op_type import AluOpType
from concourse.bass_utils import run_bass_kernel_spmd

B, S, H, R = 8, 2048, 768, 768
N_CORES = 8
P = 128          # SBUF partitions
NC_ = 512        # matmul moving free dim (one PSUM bank of fp32)
HT = H // P      # 6  k-tiles over H
RT = R // P      # 6  k-tiles over R
IT = S // P      # 16 output row tiles
SC = S // NC_    # 4  512-wide column chunks
TPC = NC_ // P   # 4  row tiles per chunk
PAIRS = HT // 2  # 3  DoubleRow k-tile pairs per 768 contraction

F32 = mybir.dt.float32
F32R = mybir.dt.float32r
BF16 = mybir.dt.bfloat16
F8 = mybir.dt.float8e4
DR = mybir.MatmulPerfMode.DoubleRow

NPF8 = ml_dtypes.float8_e4m3


def build_nc(reps=1):
    nc = bacc.Bacc("TRN2", target_bir_lowering=False, debug=False,
                   num_devices=N_CORES)

    xh_d = nc.dram_tensor("xh", [P, HT, S], F8, kind="ExternalInput")
    xl_d = nc.dram_tensor("xl", [P, HT, S], F8, kind="ExternalInput")
    ph_d = nc.dram_tensor("ph", [P, HT, R], F8, kind="ExternalInput")
    pl_d = nc.dram_tensor("pl", [P, HT, R], F8, kind="ExternalInput")
    out_d = nc.dram_tensor("out", [S, S], BF16, kind="ExternalOutput")

    with tile.TileContext(nc) as tc:
        with tc.tile_pool(name="persist", bufs=1) as sb, \
             tc.tile_pool(name="stg", bufs=4) as stg, \
             tc.tile_pool(name="pmm", bufs=2, space="PSUM") as pmm, \
             tc.tile_pool(name="pd", bufs=6, space="PSUM") as pdp:

            xh_sb = sb.tile([P, HT, S], F8, name="xh", tag="xh")
            xl_sb = sb.tile([P, HT, S], F8, name="xl", tag="xl")
            ph_sb = sb.tile([P, HT, R], F8, name="ph", tag="ph")
            pl_sb = sb.tile([P, HT, R], F8, name="pl", tag="pl")
            qq = sb.tile([P, RT, S], F8, name="qq", tag="qq")
            sqj = sb.tile([P, S], F32, name="sqj", tag="sqj")
            sqcol = sb.tile([P, IT], F32, name="sqcol", tag="sqcol")
            ident4 = sb.tile([P, NC_], F32, name="ident4", tag="id4")
            onesf = sb.tile([P, P], F32, name="onesf", tag="onesf")
            onesr = sb.tile([P, P], F32R, name="onesr", tag="onesr")

            for k in range(TPC):
                masks.make_identity(nc, ident4[:, k * P:(k + 1) * P])
            nc.vector.memset(onesf[:], 1.0)
            nc.vector.tensor_copy(onesr[:], onesf[:])

            def emit_body():
                # input loads (SP queue), chunked so compute starts early
                nc.sync.dma_start(ph_sb[:], ph_d[:, :, :])
                nc.sync.dma_start(pl_sb[:], pl_d[:, :, :])
                for c in range(SC):
                    cs = slice(c * NC_, (c + 1) * NC_)
                    nc.sync.dma_start(xh_sb[:, :, cs], xh_d[:, :, cs])
                    nc.sync.dma_start(xl_sb[:, :, cs], xl_d[:, :, cs])

                # engine rotations for the Gram epilogue. GPSIMD cannot
                # read PSUM, so the stt pass (PSUM input) lives on DVE;
                # the relu pass (SBUF->SBUF) alternates ACT/Pool.
                stt_cycle = [nc.vector] * 8
                relu_cycle = [nc.scalar, nc.gpsimd] * 4
                unit_idx = [0]

                def emit_mm(it, jc):
                    """Gram matmul group for one [128, 512] tile."""
                    js = slice(jc * NC_, (jc + 1) * NC_)
                    pd = pdp.tile([P, NC_], F32, name="pd", tag="pd")
                    for p in range(PAIRS):
                        nc.tensor.matmul(
                            pd[:],
                            qq[:, 2 * p:2 * p + 2, it * P:(it + 1) * P],
                            qq[:, 2 * p:2 * p + 2, js],
                            start=(p == 0), stop=(p == PAIRS - 1),
                            perf_mode=DR)
                    return pd

                def emit_epilogue(it, jc, pd):
                    """relu(-2*pd + sq_j + sq_i) -> bf16 -> DRAM."""
                    js = slice(jc * NC_, (jc + 1) * NC_)
                    u = unit_idx[0]
                    unit_idx[0] += 1
                    st = stg.tile([P, NC_], BF16, name="st", tag="st",
                                  bufs=4)
                    stt_cycle[u % 8].scalar_tensor_tensor(
                        st[:], pd[:], -2.0, sqj[:, js],
                        AluOpType.mult, AluOpType.add)
                    st2 = stg.tile([P, NC_], BF16, name="st2", tag="st2",
                                   bufs=6)
                    eng = relu_cycle[u % 8]
                    if eng is nc.scalar:
                        nc.scalar.activation(
                            st2[:], st[:], mybir.ActivationFunctionType.Relu,
                            bias=sqcol[:, it:it + 1], scale=1.0)
                    else:
                        eng.tensor_scalar(
                            st2[:], st[:], sqcol[:, it:it + 1], 0.0,
                            AluOpType.add, AluOpType.max)
                    nc.sync.dma_start(out_d[it * P:(it + 1) * P, js], st2[:])

                def emit_unit(it, jc):
                    emit_epilogue(it, jc, emit_mm(it, jc))

                def emit_proj_chunk(c):
                    """t' for columns chunk c -> quantized qq chunk."""
                    cs = slice(c * NC_, (c + 1) * NC_)
                    for rt in range(RT):
                        pt = pmm.tile([P, NC_], F32, name="pt", tag="pt")
                        first = True
                        for pj, xx in ((ph_sb, xh_sb), (ph_sb, xl_sb),
                                       (pl_sb, xh_sb)):
                            for p in range(PAIRS):
                                nc.tensor.matmul(
                                    pt[:],
                                    pj[:, 2 * p:2 * p + 2,
                                       rt * P:(rt + 1) * P],
                                    xx[:, 2 * p:2 * p + 2, cs],
                                    start=first,
                                    stop=(pj is pl_sb and p == PAIRS - 1),
                                    perf_mode=DR)
                                first = False
                        nc.scalar.copy(qq[:, rt, cs], pt[:])

                def emit_wave(c):
                    """All Gram tiles with max(row_chunk, col_chunk)==c.

                    The 4 diagonal-containing tiles go first; sq for
                    chunk c is extracted from their PSUM before their
                    epilogues run.
                    """
                    cs = slice(c * NC_, (c + 1) * NC_)
                    diag_pds = []
                    for k in range(TPC):
                        it = c * TPC + k
                        diag_pds.append((it, emit_mm(it, c)))
                    dm = stg.tile([P, NC_], F32R, name="dm", tag="dm",
                                  bufs=2)
                    for k, (it, pd) in enumerate(diag_pds):
                        ks = slice(k * P, (k + 1) * P)
                        nc.vector.tensor_mul(dm[:, ks], pd[:, ks],
                                             ident4[:, ks])
                        nc.vector.tensor_reduce(
                            sqcol[:, it:it + 1], dm[:, ks],
                            axis=mybir.AxisListType.X, op=AluOpType.add)
                    sq_ps = pmm.tile([P, NC_], F32, name="sqps", tag="pt")
                    nc.tensor.matmul(sq_ps[:], onesr[:], dm[:],
                                     start=True, stop=True)
                    nc.scalar.copy(sqj[:, cs], sq_ps[:])
                    for it, pd in diag_pds:
                        emit_epilogue(it, c, pd)
                    for k in range(TPC):
                        it = c * TPC + k
                        for jc in range(c):
                            emit_unit(it, jc)
                    for it in range(c * TPC):
                        emit_unit(it, c)

                # chunk-pipelined schedule: wave c is emitted after
                # projection chunk c+1 so the fp8 quantize of chunk c has
                # drained before PE reaches wave c's matmuls.
                emit_proj_chunk(0)
                for c in range(SC):
                    if c + 1 < SC:
                        emit_proj_chunk(c + 1)
                    emit_wave(c)

            for _ in range(reps):
                emit_body()

    nc.finalize()
    return nc


_NC_CACHE = {}


def get_nc(reps=1):
    key = reps
    if key not in _NC_CACHE:
        _NC_CACHE[key] = build_nc(reps)
    return _NC_CACHE[key]


def _split8(a):
    """hi/lo fp8e4 residual split of a float32 array."""
    hi = a.astype(NPF8)
    lo = (a - hi.astype(np.float32)).astype(NPF8)
    return hi, lo


def _pack(a8):
    """[H, N] -> [128, HT, N] partition-major tiling."""
    n = a8.shape[1]
    return np.ascontiguousarray(
        a8.reshape(HT, P, n).transpose(1, 0, 2))


def make_in_maps(batch, proj):
    ph, pl = _split8(np.ascontiguousarray(proj, dtype=np.float32))
    ph, pl = _pack(ph), _pack(pl)
    maps = []
    for b in range(B):
        xT = np.ascontiguousarray(batch[b].T, dtype=np.float32)
        xh, xl = _split8(xT)
        maps.append({"xh": _pack(xh), "xl": _pack(xl), "ph": ph, "pl": pl})
    return maps


def kernel(batch, proj):
    assert batch.shape == (B, S, H) and proj.shape == (H, R)
    nc = get_nc()
    in_maps = make_in_maps(batch, proj)
    res = run_bass_kernel_spmd(nc, in_maps, core_ids=list(range(N_CORES)))
    out = np.stack([np.asarray(res.results[b]["out"]) for b in range(B)],
                   axis=0)
    return out.astype(np.float32)
